# revision 19
# baseline (speedup 1.0000x reference)
"""Trainium2 Bass kernel for DeterministicTrajectoryPredictor.

Data-parallel over the fused b*o=4096 trajectory axis: 8 cores x 512
trajectories. Each core runs the full 2-layer GRU encoder (T=64 steps) and
the GRU+MLP decoder (horizon steps) for its shard; weights are replicated.

Layout: all [512, M] activations live transposed as [128, 4, M] SBUF tiles
(feature/gate dim on partitions, batch on the free axis), so each GRU matmul
is  ghT[gate, m] = sum_k WhhT[k, gate] * hT[k, m]  with no transposes in the
recurrence.  The 2->H input embedding is folded host-side into the GRU input
weights (biases are all zero in this model), making layer-0/decoder gi a K=2
matmul.  Sigmoid is computed as 0.5 + 0.5*tanh(0.5*x) so every activation in
the kernel (Tanh/Gelu/Identity/Copy) lives in the single `gelu_and_others`
ACT table set - no ~2.7us table reloads.  Encoder layer 1 runs one step
behind layer 0 (wavefront skew) so the PE never waits on a gate chain; h0 is
ping-pong buffered to keep the skew race-free.
"""

import sys

for _p in ("/opt/trn_rl_repo", "/root/.axon_site/_ro/trn_rl_repo"):
    if _p not in sys.path:
        sys.path.insert(0, _p)

import numpy as np

import concourse.bass as bass  # noqa: F401
import concourse.mybir as mybir
import concourse.tile as tile
from concourse import bacc
from concourse.bass_utils import run_bass_kernel_spmd

F32 = mybir.dt.float32
F32R = mybir.dt.float32r
AF = mybir.ActivationFunctionType
ALU = mybir.AluOpType

# Matmul input interpretation: float32r streams fp32 at 1 cycle/row (vs 4
# for float32's two half-speed passes).  Set per-region below; numerics
# validated against the fp32 jax reference on hardware.
ENC_MM_DT = F32
DEC_MM_DT = F32


def _dt_of(mm_dt):
    return F32R if mm_dt is F32R else F32


def _mm(nc, out_ap, lhsT, rhs, mm_dt, **kw):
    if mm_dt is not F32:
        lhsT = lhsT.bitcast(mm_dt)
        rhs = rhs.bitcast(mm_dt)
    nc.tensor.matmul(out_ap, lhsT, rhs, **kw)

B, T, O = 32, 64, 128
H, CTX = 512, 1024
NCORES = 8
BPC = B // NCORES          # b values per core
M = BPC * O                # trajectories per core (batch on the free axis)
HC = H // 128              # feature chunks of 128
KCTX = CTX // 128
MAX_STEP = 5.0

_CACHE = {}


def _emit_gru_cell(nc, ps, scratch, half_ap, h_in, h_out, x_rhs, wf, wih, whh,
                   r_sb, z_sb, mm_dt, gi_last=False):
    """h_out <- GRUCell(x, h_in).   h_out may alias h_in.

    x_rhs: [2, M] rhs AP when wf is given (K=2 fused-embedding path).
    wf:    [2, 3H] fused input weights, or None.
    wih:   (w_tile [128, HC, 3H], x_tile [128, HC, M]) for the K=H path.
    """
    def gi_mms(out_ap, g, start, stop):
        if wf is not None:
            _mm(nc, out_ap, wf[:, g * 128:(g + 1) * 128], x_rhs, mm_dt,
                start=start, stop=stop)
        else:
            w, x = wih
            for k in range(HC):
                _mm(nc, out_ap, w[:, k, g * 128:(g + 1) * 128],
                    x[:, k, :], mm_dt, start=(start and k == 0),
                    stop=(stop and k == HC - 1))

    def gh_mms(out_ap, g, start, stop):
        for k in range(HC):
            _mm(nc, out_ap, whh[:, k, g * 128:(g + 1) * 128],
                h_in[:, k, :], mm_dt, start=(start and k == 0),
                stop=(stop and k == HC - 1))

    # r,z gates: accumulate gi+gh for gate chunks 0..7 into 4 psum tiles.
    # gi_last: emit every gh matmul before any gi matmul, so when the gi
    # operand arrives late (decoder: state is ready ~5us after h), the PE
    # FIFO has ~27us of gh work in front of the stall point instead of 4.
    rz_ps = []
    if gi_last:
        for pt in range(4):
            p = ps.tile([128, 2, 512], F32, tag="ps")
            for j in range(2):
                gh_mms(p[:, j], pt * 2 + j, start=True, stop=False)
            rz_ps.append(p)
        for pt in range(4):
            for j in range(2):
                gi_mms(rz_ps[pt][:, j], pt * 2 + j, start=False, stop=True)
    else:
        for pt in range(4):
            p = ps.tile([128, 2, 512], F32, tag="ps")
            for j in range(2):
                g = pt * 2 + j
                gh_mms(p[:, j], g, start=True, stop=False)
                gi_mms(p[:, j], g, start=False, stop=True)
            rz_ps.append(p)
    # tanh(0.5*x) straight out of PSUM, then sigma = 0.5 + 0.5*tanh
    for pt in range(4):
        dst = r_sb if pt < 2 else z_sb
        off = (pt % 2) * 2
        nc.scalar.activation(dst[:, off:off + 2, :], rz_ps[pt][:, :, :],
                             AF.Tanh, scale=0.5)
    nc.scalar.activation(r_sb, r_sb, AF.Identity, bias=half_ap, scale=0.5)
    nc.scalar.activation(z_sb, z_sb, AF.Identity, bias=half_ap, scale=0.5)

    # n gate: inn (gi only) and hn (gh only), gate chunks 8..11
    inn_ps, hn_ps = [], []
    if gi_last:
        for pt in range(2):
            p = ps.tile([128, 2, 512], F32, tag="ps", name=f"hnp{pt}")
            for j in range(2):
                gh_mms(p[:, j], 8 + pt * 2 + j, start=True, stop=True)
            hn_ps.append(p)
        for pt in range(2):
            p = ps.tile([128, 2, 512], F32, tag="ps", name=f"innp{pt}")
            for j in range(2):
                gi_mms(p[:, j], 8 + pt * 2 + j, start=True, stop=True)
            inn_ps.append(p)
    else:
        for pt in range(2):
            p = ps.tile([128, 2, 512], F32, tag="ps")
            for j in range(2):
                gi_mms(p[:, j], 8 + pt * 2 + j, start=True, stop=True)
            inn_ps.append(p)
        for pt in range(2):
            p = ps.tile([128, 2, 512], F32, tag="ps")
            for j in range(2):
                gh_mms(p[:, j], 8 + pt * 2 + j, start=True, stop=True)
            hn_ps.append(p)

    # n = tanh(inn + r*hn)   (computed in-place in a scratch tile)
    n_sb = scratch.tile([128, HC, 512], F32, tag="scr")
    for pt in range(2):
        sl = slice(pt * 2, pt * 2 + 2)
        nc.vector.tensor_mul(n_sb[:, sl, :], r_sb[:, sl, :], hn_ps[pt][:, :, :])
    for pt in range(2):
        sl = slice(pt * 2, pt * 2 + 2)
        nc.vector.tensor_add(n_sb[:, sl, :], n_sb[:, sl, :],
                             inn_ps[pt][:, :, :])
    nc.scalar.activation(n_sb, n_sb, AF.Tanh)

    # h' = n + z*(h - n)
    d_sb = scratch.tile([128, HC, 512], F32, tag="scr")
    nc.vector.tensor_sub(d_sb, h_in, n_sb)
    nc.vector.tensor_mul(d_sb, z_sb, d_sb)
    nc.vector.tensor_add(h_out, n_sb, d_sb)


def _build_module(n_steps, horizon):
    nc = bacc.Bacc("TRN2", target_bir_lowering=False, debug=False,
                   num_devices=NCORES)

    ENC_DT = _dt_of(ENC_MM_DT)
    DEC_DT = _dt_of(DEC_MM_DT)
    H_DT = F32R if (ENC_MM_DT is F32R or DEC_MM_DT is F32R) else F32
    traj_d = nc.dram_tensor("traj", [n_steps, 2, M], ENC_DT,
                            kind="ExternalInput").ap()
    zbg_d = nc.dram_tensor("zbgT", [CTX, BPC], F32, kind="ExternalInput").ap()
    wf0_d = nc.dram_tensor("wf0T", [2, 3 * H], ENC_DT, kind="ExternalInput").ap()
    whh0_d = nc.dram_tensor("whh0T", [H, 3 * H], ENC_DT, kind="ExternalInput").ap()
    wih1_d = nc.dram_tensor("wih1T", [H, 3 * H], ENC_DT, kind="ExternalInput").ap()
    whh1_d = nc.dram_tensor("whh1T", [H, 3 * H], ENC_DT, kind="ExternalInput").ap()
    dwf_d = nc.dram_tensor("dwfT", [2, 3 * H], DEC_DT, kind="ExternalInput").ap()
    dwhh_d = nc.dram_tensor("dwhhT", [H, 3 * H], DEC_DT, kind="ExternalInput").ap()
    wctx_d = nc.dram_tensor("wctxT", [CTX, H], F32, kind="ExternalInput").ap()
    w1_d = nc.dram_tensor("w1T", [H, H], DEC_DT, kind="ExternalInput").ap()
    w2_d = nc.dram_tensor("w2T", [H, 2], DEC_DT, kind="ExternalInput").ap()
    # per-row (lat/lon) clamp+wrap constants: [clamp_lo, clamp_hi, wrap_mul,
    # wrap_add] per partition; lat row clamps, lon row wraps.
    wrapc_d = nc.dram_tensor("wrapc", [2, 4], F32, kind="ExternalInput").ap()
    out_d = nc.dram_tensor("out", [horizon, 2, M], F32,
                           kind="ExternalOutput").ap()

    with tile.TileContext(nc) as tc:
        from contextlib import ExitStack
        with ExitStack() as ctx:
            wp = ctx.enter_context(tc.tile_pool(name="weights", bufs=1))
            ps = ctx.enter_context(
                tc.tile_pool(name="psum", bufs=4, space="PSUM"))
            scratch = ctx.enter_context(tc.tile_pool(name="scratch", bufs=2))
            gates = ctx.enter_context(tc.tile_pool(name="gates", bufs=1))
            trp = ctx.enter_context(tc.tile_pool(name="traj", bufs=3))
            smalls = ctx.enter_context(tc.tile_pool(name="smalls", bufs=4))

            # ---- weights to SBUF ----
            def load_big(dram, label):
                t = wp.tile([128, HC, 3 * H], dram.dtype, name=f"w_{label}")
                nc.sync.dma_start(
                    out=t, in_=dram.rearrange("(kc p) g -> p kc g", p=128))
                return t

            wf0 = wp.tile([2, 3 * H], ENC_DT)
            nc.sync.dma_start(out=wf0, in_=wf0_d)
            dwf = wp.tile([2, 3 * H], DEC_DT)
            nc.sync.dma_start(out=dwf, in_=dwf_d)
            whh0 = load_big(whh0_d, "whh0")
            wih1 = load_big(wih1_d, "wih1")
            whh1 = load_big(whh1_d, "whh1")
            dwhh = load_big(dwhh_d, "dwhh")
            w1 = wp.tile([128, HC, H], DEC_DT)
            nc.sync.dma_start(out=w1,
                              in_=w1_d.rearrange("(kc p) g -> p kc g", p=128))
            w2 = wp.tile([128, HC, 2], DEC_DT)
            nc.sync.dma_start(out=w2,
                              in_=w2_d.rearrange("(kc p) g -> p kc g", p=128))
            zbg = wp.tile([128, KCTX, BPC], F32)
            nc.sync.dma_start(
                out=zbg, in_=zbg_d.rearrange("(kc p) b -> p kc b", p=128))
            wrapc = wp.tile([2, 4], F32)
            nc.sync.dma_start(out=wrapc, in_=wrapc_d)

            half = wp.tile([128, 1], F32)
            nc.vector.memset(half, 0.5)
            half_ap = half[:, 0:1]

            # ---- ctx = (z_bg @ W_ctx.T).T per-core slice: [H, BPC] ----
            ctx_sb = wp.tile([128, HC, BPC], F32)
            ctx_ps = [ps.tile([128, BPC], F32, tag="ps", name=f"ctxps{c}")
                      for c in range(HC)]
            for stage in range(2):  # stream W_ctx in two [512, H] halves
                wh = scratch.tile([128, HC, H], F32, tag="scr")
                nc.sync.dma_start(
                    out=wh,
                    in_=wctx_d[stage * 512:(stage + 1) * 512, :].rearrange(
                        "(kc p) g -> p kc g", p=128))
                for c in range(HC):
                    for k in range(HC):
                        kk = stage * HC + k
                        nc.tensor.matmul(ctx_ps[c],
                                         wh[:, k, c * 128:(c + 1) * 128],
                                         zbg[:, kk, :],
                                         start=(kk == 0), stop=(kk == KCTX - 1))
            for c in range(HC):
                nc.scalar.copy(ctx_sb[:, c, :], ctx_ps[c])

            # ---- persistent state ----
            h0a = wp.tile([128, HC, 512], _dt_of(ENC_MM_DT))
            h0b = wp.tile([128, HC, 512], _dt_of(ENC_MM_DT))
            h1 = wp.tile([128, HC, 512], H_DT)
            nc.vector.memset(h0a[:, :, :].bitcast(F32), 0.0)
            nc.vector.memset(h1[:, :, :].bitcast(F32), 0.0)
            h0 = [h0a, h0b]

            r_sb = gates.tile([128, HC, 512], F32, tag="r")
            z_sb = gates.tile([128, HC, 512], F32, tag="z")

            # ---- encoder: layer 1 runs one step behind layer 0 ----
            for t in range(n_steps + 1):
                if t < n_steps:
                    xt = trp.tile([2, 512], ENC_DT, tag="x")
                    nc.sync.dma_start(out=xt, in_=traj_d[t])
                    _emit_gru_cell(nc, ps, scratch, half_ap,
                                   h0[t % 2], h0[(t + 1) % 2],
                                   xt, wf0, None, whh0, r_sb, z_sb,
                                   ENC_MM_DT)
                if t >= 1:
                    _emit_gru_cell(nc, ps, scratch, half_ap,
                                   h1, h1,
                                   None, None, (wih1, h0[t % 2]), whh1,
                                   r_sb, z_sb, ENC_MM_DT)

            # ---- h_dec = h1 + ctx (broadcast over the o axis) ----
            h_dec = wp.tile([128, HC, 512], _dt_of(DEC_MM_DT))
            for c in range(HC):
                for b in range(BPC):
                    nc.vector.tensor_scalar_add(
                        h_dec[:, c, b * 128:(b + 1) * 128],
                        h1[:, c, b * 128:(b + 1) * 128],
                        ctx_sb[:, c, b:b + 1])

            # ---- decoder ----
            state = wp.tile([2, 512], _dt_of(DEC_MM_DT))
            nc.sync.dma_start(
                out=state,
                in_=traj_d[n_steps - 1].bitcast(_dt_of(DEC_MM_DT)))

            for s in range(horizon):
                _emit_gru_cell(nc, ps, scratch, half_ap, h_dec, h_dec,
                               state, dwf, None, dwhh, r_sb, z_sb,
                               DEC_MM_DT, gi_last=True)
                # y1 = gelu(h @ W1.T)
                y1_sb = scratch.tile([128, HC, 512], _dt_of(DEC_MM_DT),
                                     tag="scr")
                for pt in range(2):
                    p = ps.tile([128, 2, 512], F32, tag="ps")
                    for j in range(2):
                        c = pt * 2 + j
                        for k in range(HC):
                            _mm(nc, p[:, j],
                                w1[:, k, c * 128:(c + 1) * 128],
                                h_dec[:, k, :], DEC_MM_DT, start=(k == 0),
                                stop=(k == HC - 1))
                    nc.scalar.activation(y1_sb[:, pt * 2:pt * 2 + 2, :],
                                         p[:, :, :], AF.Gelu)
                # delta = y1 @ W2.T  -> [2, M]
                dp = ps.tile([2, 512], F32, tag="ps")
                for k in range(HC):
                    _mm(nc, dp, w2[:, k, :], y1_sb[:, k, :], DEC_MM_DT,
                        start=(k == 0), stop=(k == HC - 1))
                # delta = MAX_STEP * tanh(delta / MAX_STEP); state += delta
                dt = smalls.tile([2, 512], F32, tag="sm")
                nc.scalar.activation(dt, dp, AF.Tanh, scale=1.0 / MAX_STEP)
                nc.vector.tensor_scalar_mul(dt, dt, MAX_STEP)
                nc.vector.tensor_add(state, state, dt)
                # clamp: lat to [-90, 90]; lon row gets +-1e38 bounds (no-op)
                nc.vector.tensor_scalar(out=state, in0=state,
                                        scalar1=wrapc[:, 0:1],
                                        scalar2=wrapc[:, 1:2],
                                        op0=ALU.max, op1=ALU.min)
                # wrap lon to [0, 360) (domain (-MAX_STEP, 360+MAX_STEP)):
                #   lo = [x < 0], v = [x < 360]   (step masks via clamps)
                #   x += 360*(lo + v - 1)  ==  +360 if x<0, -360 if x>=360
                # lat row is untouched: its wrap_mul/wrap_add are 0.
                m_lo = smalls.tile([2, 512], F32, tag="sm")
                nc.vector.tensor_scalar(out=m_lo, in0=state,
                                        scalar1=-1e30, scalar2=0.0,
                                        op0=ALU.mult, op1=ALU.max)
                nc.vector.tensor_scalar(out=m_lo, in0=m_lo,
                                        scalar1=1.0, scalar2=None,
                                        op0=ALU.min)
                m_hi = smalls.tile([2, 512], F32, tag="sm")
                nc.vector.tensor_scalar(out=m_hi, in0=state,
                                        scalar1=360.0, scalar2=-1e30,
                                        op0=ALU.subtract, op1=ALU.mult)
                nc.vector.tensor_scalar(out=m_hi, in0=m_hi,
                                        scalar1=0.0, scalar2=1.0,
                                        op0=ALU.max, op1=ALU.min)
                nc.vector.tensor_add(m_lo, m_lo, m_hi)
                nc.vector.tensor_scalar(out=m_lo, in0=m_lo,
                                        scalar1=wrapc[:, 2:3],
                                        scalar2=wrapc[:, 3:4],
                                        op0=ALU.mult, op1=ALU.add)
                nc.vector.tensor_add(state, state, m_lo)
                nc.sync.dma_start(out=out_d[s], in_=state[:, :])

    nc.compile()
    return nc


def _get_module(n_steps, horizon):
    key = (n_steps, horizon)
    if key not in _CACHE:
        _CACHE[key] = _build_module(n_steps, horizon)
    return _CACHE[key]


def prep_in_maps(traj_hist, z_bg, W_embed, W_ctx, enc_Wih, enc_Whh,
                 dec_Wih, dec_Whh, W1, W2):
    n_steps = traj_hist.shape[1]
    shared = {
        "wf0T": np.ascontiguousarray((enc_Wih[0] @ W_embed).T),
        "whh0T": np.ascontiguousarray(enc_Whh[0].T),
        "wih1T": np.ascontiguousarray(enc_Wih[1].T),
        "whh1T": np.ascontiguousarray(enc_Whh[1].T),
        "dwfT": np.ascontiguousarray((dec_Wih @ W_embed).T),
        "dwhhT": np.ascontiguousarray(dec_Whh.T),
        "wctxT": np.ascontiguousarray(W_ctx.T),
        "w1T": np.ascontiguousarray(W1.T),
        "w2T": np.ascontiguousarray(W2.T),
        "wrapc": np.array([[-90.0, 90.0, 0.0, 0.0],
                           [-1e38, 1e38, 360.0, -360.0]], dtype=np.float32),
    }
    in_maps = []
    for c in range(NCORES):
        slab = traj_hist[c * BPC:(c + 1) * BPC]        # [BPC, T, O, 2]
        traj = np.ascontiguousarray(
            slab.transpose(1, 3, 0, 2).reshape(n_steps, 2, M))
        zbgT = np.ascontiguousarray(z_bg[c * BPC:(c + 1) * BPC].T)
        in_maps.append({"traj": traj, "zbgT": zbgT, **shared})
    return in_maps


def kernel(**inputs):
    traj_hist = np.asarray(inputs["traj_hist"], dtype=np.float32)
    z_bg = np.asarray(inputs["z_bg_global"], dtype=np.float32)
    horizon = int(inputs["horizon"])
    if horizon <= 0:
        return np.zeros((B, 0, O, 2), dtype=np.float32)

    nc = _get_module(T, horizon)
    in_maps = prep_in_maps(
        traj_hist, z_bg,
        np.asarray(inputs["W_embed"], dtype=np.float32),
        np.asarray(inputs["W_ctx"], dtype=np.float32),
        np.asarray(inputs["enc_Wih"], dtype=np.float32),
        np.asarray(inputs["enc_Whh"], dtype=np.float32),
        np.asarray(inputs["dec_Wih"], dtype=np.float32),
        np.asarray(inputs["dec_Whh"], dtype=np.float32),
        np.asarray(inputs["W1"], dtype=np.float32),
        np.asarray(inputs["W2"], dtype=np.float32))

    res = run_bass_kernel_spmd(nc, in_maps, core_ids=list(range(NCORES)))
    outs = np.stack([res.results[i]["out"] for i in range(NCORES)])
    # [NCORES, hz, 2, M] -> [B, hz, O, 2]
    pred = outs.reshape(NCORES, horizon, 2, BPC, O)
    pred = pred.transpose(0, 3, 1, 4, 2).reshape(B, horizon, O, 2)
    return np.ascontiguousarray(pred)


if __name__ == "__main__":
    import time
    t0 = time.time()
    _get_module(T, 32)
    print(f"build+compile: {time.time()-t0:.1f}s")


# revision 20
# speedup vs baseline: 1.1898x; 1.1898x over previous
"""Trainium2 Bass kernel for DeterministicTrajectoryPredictor.

Data-parallel over the fused b*o=4096 trajectory axis: 8 cores x 512
trajectories. Each core runs the full 2-layer GRU encoder (T=64 steps) and
the GRU+MLP decoder (horizon steps) for its shard; weights are replicated.

Layout: all [512, M] activations live transposed as [128, 4, M] SBUF tiles
(feature/gate dim on partitions, batch on the free axis), so each GRU matmul
is  ghT[gate, m] = sum_k WhhT[k, gate] * hT[k, m]  with no transposes in the
recurrence.  The 2->H input embedding is folded host-side into the GRU input
weights (biases are all zero in this model), making layer-0/decoder gi a K=2
matmul.  Sigmoid is computed as 0.5 + 0.5*tanh(0.5*x) so every activation in
the kernel (Tanh/Gelu/Identity/Copy) lives in the single `gelu_and_others`
ACT table set - no ~2.7us table reloads.  Encoder layer 1 runs one step
behind layer 0 (wavefront skew) so the PE never waits on a gate chain; h0 is
ping-pong buffered to keep the skew race-free.
"""

import sys

for _p in ("/opt/trn_rl_repo", "/root/.axon_site/_ro/trn_rl_repo"):
    if _p not in sys.path:
        sys.path.insert(0, _p)

import numpy as np

import concourse.bass as bass  # noqa: F401
import concourse.mybir as mybir
import concourse.tile as tile
from concourse import bacc
from concourse.bass_utils import run_bass_kernel_spmd

F32 = mybir.dt.float32
F32R = mybir.dt.float32r
AF = mybir.ActivationFunctionType
ALU = mybir.AluOpType

# Matmul input interpretation: float32r streams fp32 at 1 cycle/row (vs 4
# for float32's two half-speed passes).  Set per-region below; numerics
# validated against the fp32 jax reference on hardware.
ENC_MM_DT = F32
DEC_MM_DT = F32


def _dt_of(mm_dt):
    return F32R if mm_dt is F32R else F32


def _mm(nc, out_ap, lhsT, rhs, mm_dt, **kw):
    if mm_dt is not F32:
        lhsT = lhsT.bitcast(mm_dt)
        rhs = rhs.bitcast(mm_dt)
    nc.tensor.matmul(out_ap, lhsT, rhs, **kw)

B, T, O = 32, 64, 128
H, CTX = 512, 1024
NCORES = 8
BPC = B // NCORES          # b values per core
M = BPC * O                # trajectories per core (batch on the free axis)
HC = H // 128              # feature chunks of 128
KCTX = CTX // 128
MAX_STEP = 5.0

_CACHE = {}


def _emit_gru_cell(nc, ps, scratch, half_ap, h_in, h_out, x_rhs, wf, wih, whh,
                   r_sb, z_sb, mm_dt, gi_last=False):
    """h_out <- GRUCell(x, h_in).   h_out may alias h_in.

    x_rhs: [2, M] rhs AP when wf is given (K=2 fused-embedding path).
    wf:    [2, 3H] fused input weights, or None.
    wih:   (w_tile [128, HC, 3H], x_tile [128, HC, M]) for the K=H path.
    """
    def gi_mms(out_ap, g, start, stop):
        if wf is not None:
            _mm(nc, out_ap, wf[:, g * 128:(g + 1) * 128], x_rhs, mm_dt,
                start=start, stop=stop)
        else:
            w, x = wih
            for k in range(HC):
                _mm(nc, out_ap, w[:, k, g * 128:(g + 1) * 128],
                    x[:, k, :], mm_dt, start=(start and k == 0),
                    stop=(stop and k == HC - 1))

    def gh_mms(out_ap, g, start, stop):
        for k in range(HC):
            _mm(nc, out_ap, whh[:, k, g * 128:(g + 1) * 128],
                h_in[:, k, :], mm_dt, start=(start and k == 0),
                stop=(stop and k == HC - 1))

    # r,z gates: accumulate gi+gh for gate chunks 0..7 into 4 psum tiles.
    # gi_last: emit every gh matmul before any gi matmul, so when the gi
    # operand arrives late (decoder: state is ready ~5us after h), the PE
    # FIFO has ~27us of gh work in front of the stall point instead of 4.
    rz_ps = []
    if gi_last:
        for pt in range(4):
            p = ps.tile([128, 2, 512], F32, tag="ps")
            for j in range(2):
                gh_mms(p[:, j], pt * 2 + j, start=True, stop=False)
            rz_ps.append(p)
        for pt in range(4):
            for j in range(2):
                gi_mms(rz_ps[pt][:, j], pt * 2 + j, start=False, stop=True)
    else:
        for pt in range(4):
            p = ps.tile([128, 2, 512], F32, tag="ps")
            for j in range(2):
                g = pt * 2 + j
                gh_mms(p[:, j], g, start=True, stop=False)
                gi_mms(p[:, j], g, start=False, stop=True)
            rz_ps.append(p)
    # tanh(0.5*x) straight out of PSUM, then sigma = 0.5 + 0.5*tanh
    for pt in range(4):
        dst = r_sb if pt < 2 else z_sb
        off = (pt % 2) * 2
        nc.scalar.activation(dst[:, off:off + 2, :], rz_ps[pt][:, :, :],
                             AF.Tanh, scale=0.5)
    nc.scalar.activation(r_sb, r_sb, AF.Identity, bias=half_ap, scale=0.5)
    nc.scalar.activation(z_sb, z_sb, AF.Identity, bias=half_ap, scale=0.5)

    # n gate: inn (gi only) and hn (gh only), gate chunks 8..11
    inn_ps, hn_ps = [], []
    if gi_last:
        for pt in range(2):
            p = ps.tile([128, 2, 512], F32, tag="ps", name=f"hnp{pt}")
            for j in range(2):
                gh_mms(p[:, j], 8 + pt * 2 + j, start=True, stop=True)
            hn_ps.append(p)
        for pt in range(2):
            p = ps.tile([128, 2, 512], F32, tag="ps", name=f"innp{pt}")
            for j in range(2):
                gi_mms(p[:, j], 8 + pt * 2 + j, start=True, stop=True)
            inn_ps.append(p)
    else:
        for pt in range(2):
            p = ps.tile([128, 2, 512], F32, tag="ps")
            for j in range(2):
                gi_mms(p[:, j], 8 + pt * 2 + j, start=True, stop=True)
            inn_ps.append(p)
        for pt in range(2):
            p = ps.tile([128, 2, 512], F32, tag="ps")
            for j in range(2):
                gh_mms(p[:, j], 8 + pt * 2 + j, start=True, stop=True)
            hn_ps.append(p)

    # n = tanh(inn + r*hn);  h' = n + z*(h - n)
    n_sb = scratch.tile([128, HC, 512], F32, tag="scr")
    d_sb = scratch.tile([128, HC, 512], F32, tag="scr")
    if gi_last:
        # decoder: per-chunk chain so the first h chunk lands ~3.5us after
        # the last n matmul -- W1/next-step matmuls start early and PE-idle
        # gaps stay under the ~3.4us HAM re-throttle window.
        for c in range(HC):
            nc.vector.tensor_mul(n_sb[:, c, :], r_sb[:, c, :],
                                 hn_ps[c // 2][:, c % 2, :])
            nc.vector.tensor_add(n_sb[:, c, :], n_sb[:, c, :],
                                 inn_ps[c // 2][:, c % 2, :])
            nc.scalar.activation(n_sb[:, c, :], n_sb[:, c, :], AF.Tanh)
            nc.vector.tensor_sub(d_sb[:, c, :], h_in[:, c, :], n_sb[:, c, :])
            nc.vector.tensor_mul(d_sb[:, c, :], z_sb[:, c, :], d_sb[:, c, :])
            nc.vector.tensor_add(h_out[:, c, :], n_sb[:, c, :], d_sb[:, c, :])
    else:
        for pt in range(2):
            sl = slice(pt * 2, pt * 2 + 2)
            nc.vector.tensor_mul(n_sb[:, sl, :], r_sb[:, sl, :],
                                 hn_ps[pt][:, :, :])
        for pt in range(2):
            sl = slice(pt * 2, pt * 2 + 2)
            nc.vector.tensor_add(n_sb[:, sl, :], n_sb[:, sl, :],
                                 inn_ps[pt][:, :, :])
        nc.scalar.activation(n_sb, n_sb, AF.Tanh)
        nc.vector.tensor_sub(d_sb, h_in, n_sb)
        nc.vector.tensor_mul(d_sb, z_sb, d_sb)
        nc.vector.tensor_add(h_out, n_sb, d_sb)


def _build_module(n_steps, horizon):
    nc = bacc.Bacc("TRN2", target_bir_lowering=False, debug=False,
                   num_devices=NCORES)

    ENC_DT = _dt_of(ENC_MM_DT)
    DEC_DT = _dt_of(DEC_MM_DT)
    H_DT = F32R if (ENC_MM_DT is F32R or DEC_MM_DT is F32R) else F32
    traj_d = nc.dram_tensor("traj", [n_steps, 2, M], ENC_DT,
                            kind="ExternalInput").ap()
    zbg_d = nc.dram_tensor("zbgT", [CTX, BPC], F32, kind="ExternalInput").ap()
    wf0_d = nc.dram_tensor("wf0T", [2, 3 * H], ENC_DT, kind="ExternalInput").ap()
    whh0_d = nc.dram_tensor("whh0T", [H, 3 * H], ENC_DT, kind="ExternalInput").ap()
    wih1_d = nc.dram_tensor("wih1T", [H, 3 * H], ENC_DT, kind="ExternalInput").ap()
    whh1_d = nc.dram_tensor("whh1T", [H, 3 * H], ENC_DT, kind="ExternalInput").ap()
    dwf_d = nc.dram_tensor("dwfT", [2, 3 * H], DEC_DT, kind="ExternalInput").ap()
    dwhh_d = nc.dram_tensor("dwhhT", [H, 3 * H], DEC_DT, kind="ExternalInput").ap()
    wctx_d = nc.dram_tensor("wctxT", [CTX, H], F32, kind="ExternalInput").ap()
    w1_d = nc.dram_tensor("w1T", [H, H], DEC_DT, kind="ExternalInput").ap()
    w2_d = nc.dram_tensor("w2T", [H, 2], DEC_DT, kind="ExternalInput").ap()
    # per-row (lat/lon) clamp+wrap constants: [clamp_lo, clamp_hi, wrap_mul,
    # wrap_add] per partition; lat row clamps, lon row wraps.
    wrapc_d = nc.dram_tensor("wrapc", [2, 4], F32, kind="ExternalInput").ap()
    out_d = nc.dram_tensor("out", [horizon, 2, M], F32,
                           kind="ExternalOutput").ap()

    with tile.TileContext(nc) as tc:
        from contextlib import ExitStack
        with ExitStack() as ctx:
            wp = ctx.enter_context(tc.tile_pool(name="weights", bufs=1))
            ps = ctx.enter_context(
                tc.tile_pool(name="psum", bufs=4, space="PSUM"))
            scratch = ctx.enter_context(tc.tile_pool(name="scratch", bufs=2))
            gates = ctx.enter_context(tc.tile_pool(name="gates", bufs=1))
            trp = ctx.enter_context(tc.tile_pool(name="traj", bufs=3))
            smalls = ctx.enter_context(tc.tile_pool(name="smalls", bufs=4))

            # ---- weights to SBUF ----
            def load_big(dram, label):
                t = wp.tile([128, HC, 3 * H], dram.dtype, name=f"w_{label}")
                nc.sync.dma_start(
                    out=t, in_=dram.rearrange("(kc p) g -> p kc g", p=128))
                return t

            wf0 = wp.tile([2, 3 * H], ENC_DT)
            nc.sync.dma_start(out=wf0, in_=wf0_d)
            dwf = wp.tile([2, 3 * H], DEC_DT)
            nc.sync.dma_start(out=dwf, in_=dwf_d)
            whh0 = load_big(whh0_d, "whh0")
            wih1 = load_big(wih1_d, "wih1")
            whh1 = load_big(whh1_d, "whh1")
            dwhh = load_big(dwhh_d, "dwhh")
            w1 = wp.tile([128, HC, H], DEC_DT)
            nc.sync.dma_start(out=w1,
                              in_=w1_d.rearrange("(kc p) g -> p kc g", p=128))
            w2 = wp.tile([128, HC, 2], DEC_DT)
            nc.sync.dma_start(out=w2,
                              in_=w2_d.rearrange("(kc p) g -> p kc g", p=128))
            zbg = wp.tile([128, KCTX, BPC], F32)
            nc.sync.dma_start(
                out=zbg, in_=zbg_d.rearrange("(kc p) b -> p kc b", p=128))
            wrapc = wp.tile([2, 4], F32)
            nc.sync.dma_start(out=wrapc, in_=wrapc_d)

            half = wp.tile([128, 1], F32)
            nc.vector.memset(half, 0.5)
            half_ap = half[:, 0:1]

            # ---- ctx = (z_bg @ W_ctx.T).T per-core slice: [H, BPC] ----
            ctx_sb = wp.tile([128, HC, BPC], F32)
            ctx_ps = [ps.tile([128, BPC], F32, tag="ps", name=f"ctxps{c}")
                      for c in range(HC)]
            for stage in range(2):  # stream W_ctx in two [512, H] halves
                wh = scratch.tile([128, HC, H], F32, tag="scr")
                nc.sync.dma_start(
                    out=wh,
                    in_=wctx_d[stage * 512:(stage + 1) * 512, :].rearrange(
                        "(kc p) g -> p kc g", p=128))
                for c in range(HC):
                    for k in range(HC):
                        kk = stage * HC + k
                        nc.tensor.matmul(ctx_ps[c],
                                         wh[:, k, c * 128:(c + 1) * 128],
                                         zbg[:, kk, :],
                                         start=(kk == 0), stop=(kk == KCTX - 1))
            for c in range(HC):
                nc.scalar.copy(ctx_sb[:, c, :], ctx_ps[c])

            # ---- persistent state ----
            h0a = wp.tile([128, HC, 512], _dt_of(ENC_MM_DT))
            h0b = wp.tile([128, HC, 512], _dt_of(ENC_MM_DT))
            h1 = wp.tile([128, HC, 512], H_DT)
            nc.vector.memset(h0a[:, :, :].bitcast(F32), 0.0)
            nc.vector.memset(h1[:, :, :].bitcast(F32), 0.0)
            h0 = [h0a, h0b]

            r_sb = gates.tile([128, HC, 512], F32, tag="r")
            z_sb = gates.tile([128, HC, 512], F32, tag="z")

            # ---- encoder: layer 1 runs one step behind layer 0 ----
            for t in range(n_steps + 1):
                if t < n_steps:
                    xt = trp.tile([2, 512], ENC_DT, tag="x")
                    nc.sync.dma_start(out=xt, in_=traj_d[t])
                    _emit_gru_cell(nc, ps, scratch, half_ap,
                                   h0[t % 2], h0[(t + 1) % 2],
                                   xt, wf0, None, whh0, r_sb, z_sb,
                                   ENC_MM_DT)
                if t >= 1:
                    _emit_gru_cell(nc, ps, scratch, half_ap,
                                   h1, h1,
                                   None, None, (wih1, h0[t % 2]), whh1,
                                   r_sb, z_sb, ENC_MM_DT)

            # ---- h_dec = h1 + ctx (broadcast over the o axis) ----
            h_dec = wp.tile([128, HC, 512], _dt_of(DEC_MM_DT))
            for c in range(HC):
                for b in range(BPC):
                    nc.vector.tensor_scalar_add(
                        h_dec[:, c, b * 128:(b + 1) * 128],
                        h1[:, c, b * 128:(b + 1) * 128],
                        ctx_sb[:, c, b:b + 1])

            # ---- decoder ----
            state = wp.tile([2, 512], _dt_of(DEC_MM_DT))
            nc.sync.dma_start(
                out=state,
                in_=traj_d[n_steps - 1].bitcast(_dt_of(DEC_MM_DT)))

            for s in range(horizon):
                _emit_gru_cell(nc, ps, scratch, half_ap, h_dec, h_dec,
                               state, dwf, None, dwhh, r_sb, z_sb,
                               DEC_MM_DT, gi_last=True)
                # y1 = gelu(h @ W1.T)
                y1_sb = scratch.tile([128, HC, 512], _dt_of(DEC_MM_DT),
                                     tag="scr")
                for pt in range(2):
                    p = ps.tile([128, 2, 512], F32, tag="ps")
                    for j in range(2):
                        c = pt * 2 + j
                        for k in range(HC):
                            _mm(nc, p[:, j],
                                w1[:, k, c * 128:(c + 1) * 128],
                                h_dec[:, k, :], DEC_MM_DT, start=(k == 0),
                                stop=(k == HC - 1))
                    nc.scalar.activation(y1_sb[:, pt * 2:pt * 2 + 2, :],
                                         p[:, :, :], AF.Gelu)
                # delta = y1 @ W2.T  -> [2, M]
                dp = ps.tile([2, 512], F32, tag="ps")
                for k in range(HC):
                    _mm(nc, dp, w2[:, k, :], y1_sb[:, k, :], DEC_MM_DT,
                        start=(k == 0), stop=(k == HC - 1))
                # delta = MAX_STEP * tanh(delta / MAX_STEP); state += delta
                dt = smalls.tile([2, 512], F32, tag="sm")
                nc.scalar.activation(dt, dp, AF.Tanh, scale=1.0 / MAX_STEP)
                nc.vector.tensor_scalar_mul(dt, dt, MAX_STEP)
                nc.vector.tensor_add(state, state, dt)
                # clamp: lat to [-90, 90]; lon row gets +-1e38 bounds (no-op)
                nc.vector.tensor_scalar(out=state, in0=state,
                                        scalar1=wrapc[:, 0:1],
                                        scalar2=wrapc[:, 1:2],
                                        op0=ALU.max, op1=ALU.min)
                # wrap lon to [0, 360) (domain (-MAX_STEP, 360+MAX_STEP)):
                #   lo = [x < 0], v = [x < 360]   (step masks via clamps)
                #   x += 360*(lo + v - 1)  ==  +360 if x<0, -360 if x>=360
                # lat row is untouched: its wrap_mul/wrap_add are 0.
                m_lo = smalls.tile([2, 512], F32, tag="sm")
                nc.vector.tensor_scalar(out=m_lo, in0=state,
                                        scalar1=-1e30, scalar2=0.0,
                                        op0=ALU.mult, op1=ALU.max)
                nc.vector.tensor_scalar(out=m_lo, in0=m_lo,
                                        scalar1=1.0, scalar2=None,
                                        op0=ALU.min)
                m_hi = smalls.tile([2, 512], F32, tag="sm")
                nc.vector.tensor_scalar(out=m_hi, in0=state,
                                        scalar1=360.0, scalar2=-1e30,
                                        op0=ALU.subtract, op1=ALU.mult)
                nc.vector.tensor_scalar(out=m_hi, in0=m_hi,
                                        scalar1=0.0, scalar2=1.0,
                                        op0=ALU.max, op1=ALU.min)
                nc.vector.tensor_add(m_lo, m_lo, m_hi)
                nc.vector.tensor_scalar(out=m_lo, in0=m_lo,
                                        scalar1=wrapc[:, 2:3],
                                        scalar2=wrapc[:, 3:4],
                                        op0=ALU.mult, op1=ALU.add)
                nc.vector.tensor_add(state, state, m_lo)
                nc.sync.dma_start(out=out_d[s], in_=state[:, :])

    nc.compile()
    return nc


def _get_module(n_steps, horizon):
    key = (n_steps, horizon)
    if key not in _CACHE:
        _CACHE[key] = _build_module(n_steps, horizon)
    return _CACHE[key]


def prep_in_maps(traj_hist, z_bg, W_embed, W_ctx, enc_Wih, enc_Whh,
                 dec_Wih, dec_Whh, W1, W2):
    n_steps = traj_hist.shape[1]
    shared = {
        "wf0T": np.ascontiguousarray((enc_Wih[0] @ W_embed).T),
        "whh0T": np.ascontiguousarray(enc_Whh[0].T),
        "wih1T": np.ascontiguousarray(enc_Wih[1].T),
        "whh1T": np.ascontiguousarray(enc_Whh[1].T),
        "dwfT": np.ascontiguousarray((dec_Wih @ W_embed).T),
        "dwhhT": np.ascontiguousarray(dec_Whh.T),
        "wctxT": np.ascontiguousarray(W_ctx.T),
        "w1T": np.ascontiguousarray(W1.T),
        "w2T": np.ascontiguousarray(W2.T),
        "wrapc": np.array([[-90.0, 90.0, 0.0, 0.0],
                           [-1e38, 1e38, 360.0, -360.0]], dtype=np.float32),
    }
    in_maps = []
    for c in range(NCORES):
        slab = traj_hist[c * BPC:(c + 1) * BPC]        # [BPC, T, O, 2]
        traj = np.ascontiguousarray(
            slab.transpose(1, 3, 0, 2).reshape(n_steps, 2, M))
        zbgT = np.ascontiguousarray(z_bg[c * BPC:(c + 1) * BPC].T)
        in_maps.append({"traj": traj, "zbgT": zbgT, **shared})
    return in_maps


def kernel(**inputs):
    traj_hist = np.asarray(inputs["traj_hist"], dtype=np.float32)
    z_bg = np.asarray(inputs["z_bg_global"], dtype=np.float32)
    horizon = int(inputs["horizon"])
    if horizon <= 0:
        return np.zeros((B, 0, O, 2), dtype=np.float32)

    nc = _get_module(T, horizon)
    in_maps = prep_in_maps(
        traj_hist, z_bg,
        np.asarray(inputs["W_embed"], dtype=np.float32),
        np.asarray(inputs["W_ctx"], dtype=np.float32),
        np.asarray(inputs["enc_Wih"], dtype=np.float32),
        np.asarray(inputs["enc_Whh"], dtype=np.float32),
        np.asarray(inputs["dec_Wih"], dtype=np.float32),
        np.asarray(inputs["dec_Whh"], dtype=np.float32),
        np.asarray(inputs["W1"], dtype=np.float32),
        np.asarray(inputs["W2"], dtype=np.float32))

    res = run_bass_kernel_spmd(nc, in_maps, core_ids=list(range(NCORES)))
    outs = np.stack([res.results[i]["out"] for i in range(NCORES)])
    # [NCORES, hz, 2, M] -> [B, hz, O, 2]
    pred = outs.reshape(NCORES, horizon, 2, BPC, O)
    pred = pred.transpose(0, 3, 1, 4, 2).reshape(B, horizon, O, 2)
    return np.ascontiguousarray(pred)


if __name__ == "__main__":
    import time
    t0 = time.time()
    _get_module(T, 32)
    print(f"build+compile: {time.time()-t0:.1f}s")
